# revision 27
# baseline (speedup 1.0000x reference)
"""Bahdanau attention Trainium2 kernel.

Full-input contract: kernel(**inputs) takes the unsharded inputs
(query [32,1,1024], keys [32,2048,1024], Wa_w/Wa_b/Ua_w/Ua_b/Va_w/Va_b)
and returns (context [32,1,1024], weights [32,1,2048]) as float32,
matching reference().

Strategy: data-parallel over batch across 8 NeuronCores (4 batches per
core). Host pre-transposes keys to [b, h, s] (fp32) so the dominant
matmul k_proj^T = Ua^T @ keys^T streams directly from DRAM with no
on-device transposes. All matmuls run in float32r (full fp32 data,
mantissa-rounded by casting gpsimd DMAs / ACT outputs), which the PE
executes at bf16 speed (1 cycle/row at N=512) with ~13x better
accuracy than bf16. Per core:
  stage 1 (PE):  e^T[h,s] = tanh(Ua^T keys^T + (q W_a + Wa_b + Ua_b))
                 f32r matmuls, fp32 PSUM accum, tanh+bias fused on ACT
  stage 2 (PE):  scores[1,s] = Va^T e^T accumulated over h chunks
  softmax (DVE/ACT): on scores row, fp32
  stage 3 (DVE): context^T[h,1] = sum_s keysT[h,s] * wexp[s] / sum via a
                 single fused scalar_tensor_tensor (multiply+reduce) pass
                 per h-chunk against the DMA-broadcast exp'd scores;
                 keysT is re-streamed from DRAM in fp32
Va_b is dropped: softmax is shift-invariant so it cancels exactly.
"""

import os

os.environ.setdefault("JAX_PLATFORMS", "axon")

import numpy as np
from contextlib import ExitStack

import concourse.bass as bass
import concourse.bacc as bacc
import concourse.tile as tile
from concourse import mybir

B, S, H = 32, 2048, 1024
NCORES = 8
BL = B // NCORES  # batches per core
P = 128
HC = H // P  # h chunks
ST = 512  # s tile for stage 1/2
NST = S // ST
F32 = mybir.dt.float32
F32R = mybir.dt.float32r

E_DTYPE = F32R  # tanh output / stage-2 operand dtype


def build_core_program(tc, outs, ins):
    """Emit the per-core Tile program.

    outs: dict with APs ctx_out [BL, H] f32, w_out [BL, S] f32
    ins:  dict with APs keysT [BL, H, S] f32, ua [H, H] f32,
          wa [H, H] f32, va [H, 1] f32, qT [H, BL] f32,
          bcomb [H] f32 (Wa_b + Ua_b)
    """
    nc = tc.nc
    Tanh = mybir.ActivationFunctionType.Tanh
    Exp = mybir.ActivationFunctionType.Exp
    add = mybir.AluOpType.add
    mult = mybir.AluOpType.mult
    amax = mybir.AluOpType.max

    # Sim harness declares plain-f32 DRAM inputs; view them as f32r so
    # HWDGE DMAs into f32r tiles are cast-free in both paths.
    ins = dict(ins)
    for name in ("keysT", "ua", "wa", "va", "qT"):
        if ins[name].dtype != F32R:
            ins[name] = ins[name].bitcast(F32R)

    with ExitStack() as ctx:
        consts = ctx.enter_context(tc.tile_pool(name="consts", bufs=1))
        wap = ctx.enter_context(tc.tile_pool(name="wap", bufs=2))
        ktp = ctx.enter_context(tc.tile_pool(name="ktp", bufs=3))
        kt3p = ctx.enter_context(tc.tile_pool(name="kt3p", bufs=3))
        epool = ctx.enter_context(tc.tile_pool(name="epool", bufs=3))
        wbcp = ctx.enter_context(tc.tile_pool(name="wbcp", bufs=2))
        smx = ctx.enter_context(tc.tile_pool(name="smx", bufs=2))
        dramp = ctx.enter_context(tc.tile_pool(name="dramp", bufs=2, space="DRAM"))
        psum_e = ctx.enter_context(tc.tile_pool(name="psum_e", bufs=4, space="PSUM"))
        psum_s = ctx.enter_context(tc.tile_pool(name="psum_s", bufs=2, space="PSUM"))
        psum_q = ctx.enter_context(tc.tile_pool(name="psum_q", bufs=2, space="PSUM"))

        keysT = ins["keysT"].rearrange("b (c p) s -> p b c s", p=P)

        # ---- constants: weights in chunked [p, chunk, ...] layout.
        # Matmul operand DRAM tensors are declared float32r (same bits as
        # the host's fp32) so plain HWDGE DMAs satisfy the BIR verifier's
        # f32r rounding chain without slow casting SWDGE transfers.
        # ua is loaded in 8 column-block DMAs so stage-1 co=0 can start
        # before the whole 4 MiB tensor lands.
        va_sb = consts.tile([P, HC, 1], F32R)
        nc.sync.dma_start(out=va_sb, in_=ins["va"].rearrange("(c p) o -> p c o", p=P))
        qt_sb = consts.tile([P, HC, BL], F32R)
        nc.sync.dma_start(out=qt_sb, in_=ins["qT"].rearrange("(c p) b -> p c b", p=P))
        bc_sb = consts.tile([P, HC], F32)
        nc.sync.dma_start(out=bc_sb, in_=ins["bcomb"].rearrange("(c p) -> p c", p=P))

        # ---- q_proj for all local batches; bias[p, co, b] = qW[co*P+p, b] + bcomb
        wa_ap = ins["wa"].rearrange("(c p) j -> p c j", p=P)
        bias_sb = consts.tile([P, HC, BL], F32)
        for co in range(HC):
            wa_co = wap.tile([P, HC, P], F32R)
            nc.sync.dma_start(out=wa_co, in_=wa_ap[:, :, co * P : (co + 1) * P])
            pq = psum_q.tile([P, BL], F32)
            for ci in range(HC):
                nc.tensor.matmul(
                    pq,
                    lhsT=wa_co[:, ci, :],
                    rhs=qt_sb[:, ci, :],
                    start=(ci == 0),
                    stop=(ci == HC - 1),
                )
            nc.vector.tensor_scalar_add(bias_sb[:, co, :], pq, bc_sb[:, co : co + 1])

        # First stage-1 keys tile right after the q_proj operands, then ua
        # in 8 column-block DMAs so stage-1 co=0 can start before the whole
        # 4 MiB tensor lands.
        kt_first = ktp.tile([P, HC, ST], F32R)
        nc.sync.dma_start(out=kt_first, in_=keysT[:, 0, :, 0:ST])
        ua_ap = ins["ua"].rearrange("(c p) j -> p c j", p=P)
        ua_sb = consts.tile([P, HC, H], F32R)
        for co in range(HC):
            nc.sync.dma_start(
                out=ua_sb[:, :, co * P : (co + 1) * P],
                in_=ua_ap[:, :, co * P : (co + 1) * P],
            )

        for b in range(BL):
            # ---- stage 1+2: e^T = tanh(Ua^T K^T + bias); scores = Va^T e^T
            sc_sb = smx.tile([1, S], F32)
            mx4 = smx.tile([1, NST], F32)
            for st in range(NST):
                sl = slice(st * ST, (st + 1) * ST)
                if b == 0 and st == 0:
                    kt = kt_first
                else:
                    kt = ktp.tile([P, HC, ST], F32R)
                    nc.sync.dma_start(out=kt, in_=keysT[:, b, :, sl])
                ps = psum_s.tile([1, ST], F32)
                for co in range(HC):
                    pe = psum_e.tile([P, ST], F32)
                    for ci in range(HC):
                        nc.tensor.matmul(
                            pe,
                            lhsT=ua_sb[:, ci, co * P : (co + 1) * P],
                            rhs=kt[:, ci, :],
                            start=(ci == 0),
                            stop=(ci == HC - 1),
                        )
                    e_sb = epool.tile([P, ST], E_DTYPE)
                    nc.scalar.activation(
                        out=e_sb, in_=pe, func=Tanh, bias=bias_sb[:, co, b : b + 1]
                    )
                    nc.tensor.matmul(
                        ps,
                        lhsT=va_sb[:, co, :],
                        rhs=e_sb,
                        start=(co == 0),
                        stop=(co == HC - 1),
                    )
                nc.vector.tensor_copy(out=sc_sb[:, sl], in_=ps)
                # partial max per s-tile, off the softmax critical path
                nc.vector.tensor_reduce(
                    mx4[:, st : st + 1], ps, axis=mybir.AxisListType.X, op=amax
                )

            # ---- softmax on the [1, S] scores row (fp32, in-place)
            mx = smx.tile([1, 1], F32)
            nc.vector.tensor_reduce(mx, mx4, axis=mybir.AxisListType.X, op=amax)
            neg_mx = smx.tile([1, 1], F32)
            nc.scalar.mul(neg_mx, mx, -1.0)
            ssum = smx.tile([1, 1], F32)
            nc.scalar.activation(
                out=sc_sb, in_=sc_sb, func=Exp, bias=neg_mx, accum_out=ssum
            )

            # un-normalized exp row -> DRAM bounce -> broadcast to 128
            # partitions; the 1/sum scale is applied to the context at the
            # end instead, keeping the reciprocal off stage-3's critical path
            wd = dramp.tile([1, S], F32)
            nc.scalar.dma_start(out=wd, in_=sc_sb)
            wbc = wbcp.tile([P, S], F32)
            nc.scalar.dma_start(out=wbc, in_=wd.broadcast_to([P, S]))

            rsum = smx.tile([1, 1], F32)
            nc.vector.reciprocal(rsum, ssum)
            rd = dramp.tile([1, 1], F32)
            nc.scalar.dma_start(out=rd, in_=rsum)
            rs_bc = smx.tile([P, 1], F32)
            nc.scalar.dma_start(out=rs_bc, in_=rd.broadcast_to([P, 1]))

            # normalized weights output row
            nc.vector.tensor_scalar_mul(sc_sb, sc_sb, rsum)
            nc.scalar.dma_start(out=outs["w_out"][b : b + 1, :], in_=sc_sb)

            # ---- stage 3: context^T[h] = sum_s keysT[h, s] * wexp[s] / sum
            # keysT re-streamed from DRAM per h-chunk in fp32; fused
            # multiply+reduce in one DVE pass via scalar_tensor_tensor
            ctxt_raw = smx.tile([P, HC], F32)
            for c in range(HC):
                kt3 = kt3p.tile([P, S], F32)
                nc.sync.dma_start(out=kt3, in_=keysT[:, b, c, :].bitcast(F32))
                junk = wbcp.tile([P, S], mybir.dt.bfloat16)
                nc.vector.scalar_tensor_tensor(
                    out=junk,
                    in0=kt3,
                    scalar=1.0,
                    in1=wbc,
                    op0=mult,
                    op1=mult,
                    accum_out=ctxt_raw[:, c : c + 1],
                )
            ctxt = smx.tile([P, HC], F32)
            nc.vector.tensor_scalar_mul(ctxt, ctxt_raw, rs_bc)
            nc.scalar.dma_start(
                out=outs["ctx_out"][b].rearrange("(c p) -> p c", p=P), in_=ctxt
            )


def build_nc():
    nc = bacc.Bacc("TRN2", target_bir_lowering=False, debug=False)
    ins = {
        "keysT": nc.dram_tensor("keysT", [BL, H, S], F32R, kind="ExternalInput"),
        "ua": nc.dram_tensor("ua", [H, H], F32R, kind="ExternalInput"),
        "wa": nc.dram_tensor("wa", [H, H], F32R, kind="ExternalInput"),
        "va": nc.dram_tensor("va", [H, 1], F32R, kind="ExternalInput"),
        "qT": nc.dram_tensor("qT", [H, BL], F32R, kind="ExternalInput"),
        "bcomb": nc.dram_tensor("bcomb", [H], F32, kind="ExternalInput"),
    }
    outs = {
        "ctx_out": nc.dram_tensor("ctx_out", [BL, H], F32, kind="ExternalOutput"),
        "w_out": nc.dram_tensor("w_out", [BL, S], F32, kind="ExternalOutput"),
    }
    with tile.TileContext(nc) as tc:
        build_core_program(
            tc,
            {k: v.ap() for k, v in outs.items()},
            {k: v.ap() for k, v in ins.items()},
        )
    nc.compile()
    return nc


_NC_CACHE = []


def prepare_in_maps(query, keys, Wa_w, Wa_b, Ua_w, Ua_b, Va_w, Va_b):
    query = np.asarray(query, dtype=np.float32)
    keys = np.asarray(keys, dtype=np.float32)
    ua_h = np.ascontiguousarray(np.asarray(Ua_w, dtype=np.float32))
    wa_h = np.ascontiguousarray(np.asarray(Wa_w, dtype=np.float32))
    va_h = np.ascontiguousarray(np.asarray(Va_w, dtype=np.float32))
    bcomb = (np.asarray(Wa_b, np.float32) + np.asarray(Ua_b, np.float32)).astype(
        np.float32
    )
    in_maps = []
    for c in range(NCORES):
        bs = slice(c * BL, (c + 1) * BL)
        in_maps.append(
            {
                "keysT": np.ascontiguousarray(keys[bs].transpose(0, 2, 1)),
                "ua": ua_h,
                "wa": wa_h,
                "va": va_h,
                "qT": np.ascontiguousarray(query[bs, 0, :].T),
                "bcomb": bcomb,
            }
        )
    return in_maps


def kernel(query, keys, Wa_w, Wa_b, Ua_w, Ua_b, Va_w, Va_b):
    from concourse.bass_utils import run_bass_kernel_spmd

    if not _NC_CACHE:
        _NC_CACHE.append(build_nc())
    nc = _NC_CACHE[0]
    in_maps = prepare_in_maps(query, keys, Wa_w, Wa_b, Ua_w, Ua_b, Va_w, Va_b)
    res = run_bass_kernel_spmd(nc, in_maps, list(range(NCORES)))
    ctx = np.concatenate([np.asarray(r["ctx_out"]) for r in res.results], axis=0)
    wts = np.concatenate([np.asarray(r["w_out"]) for r in res.results], axis=0)
    return (
        ctx.reshape(B, 1, H).astype(np.float32),
        wts.reshape(B, 1, S).astype(np.float32),
    )


# revision 28
# speedup vs baseline: 1.0078x; 1.0078x over previous
"""Bahdanau attention Trainium2 kernel.

Full-input contract: kernel(**inputs) takes the unsharded inputs
(query [32,1,1024], keys [32,2048,1024], Wa_w/Wa_b/Ua_w/Ua_b/Va_w/Va_b)
and returns (context [32,1,1024], weights [32,1,2048]) as float32,
matching reference().

Strategy: data-parallel over batch across 8 NeuronCores (4 batches per
core). Host pre-transposes keys to [b, h, s] (fp32) so the dominant
matmul k_proj^T = Ua^T @ keys^T streams directly from DRAM with no
on-device transposes. All matmuls run in float32r (full fp32 data,
mantissa-rounded by casting gpsimd DMAs / ACT outputs), which the PE
executes at bf16 speed (1 cycle/row at N=512) with ~13x better
accuracy than bf16. Per core:
  stage 1 (PE):  e^T[h,s] = tanh(Ua^T keys^T + (q W_a + Wa_b + Ua_b))
                 f32r matmuls, fp32 PSUM accum, tanh+bias fused on ACT
  stage 2 (PE):  scores[1,s] = Va^T e^T accumulated over h chunks
  softmax (DVE/ACT): on scores row, fp32
  stage 3 (DVE): context^T[h,1] = sum_s keysT[h,s] * wexp[s] / sum via a
                 single fused scalar_tensor_tensor (multiply+reduce) pass
                 per h-chunk against the DMA-broadcast exp'd scores;
                 keysT is re-streamed from DRAM in fp32
Va_b is dropped: softmax is shift-invariant so it cancels exactly.
"""

import os

os.environ.setdefault("JAX_PLATFORMS", "axon")

import numpy as np
from contextlib import ExitStack

import concourse.bass as bass
import concourse.bacc as bacc
import concourse.tile as tile
from concourse import mybir

B, S, H = 32, 2048, 1024
NCORES = 8
BL = B // NCORES  # batches per core
P = 128
HC = H // P  # h chunks
ST = 512  # s tile for stage 1/2
NST = S // ST
F32 = mybir.dt.float32
F32R = mybir.dt.float32r

E_DTYPE = F32R  # tanh output / stage-2 operand dtype


def build_core_program(tc, outs, ins):
    """Emit the per-core Tile program.

    outs: dict with APs ctx_out [BL, H] f32, w_out [BL, S] f32
    ins:  dict with APs keysT [BL, H, S] f32, ua [H, H] f32,
          wa [H, H] f32, va [H, 1] f32, qT [H, BL] f32,
          bcomb [H] f32 (Wa_b + Ua_b)
    """
    nc = tc.nc
    Tanh = mybir.ActivationFunctionType.Tanh
    Exp = mybir.ActivationFunctionType.Exp
    add = mybir.AluOpType.add
    mult = mybir.AluOpType.mult
    amax = mybir.AluOpType.max

    # Sim harness declares plain-f32 DRAM inputs; view them as f32r so
    # HWDGE DMAs into f32r tiles are cast-free in both paths.
    ins = dict(ins)
    for name in ("keysT", "ua", "wa", "va", "qT"):
        if ins[name].dtype != F32R:
            ins[name] = ins[name].bitcast(F32R)

    with ExitStack() as ctx:
        consts = ctx.enter_context(tc.tile_pool(name="consts", bufs=1))
        wap = ctx.enter_context(tc.tile_pool(name="wap", bufs=2))
        ktp = ctx.enter_context(tc.tile_pool(name="ktp", bufs=3))
        kt3p = ctx.enter_context(tc.tile_pool(name="kt3p", bufs=3))
        epool = ctx.enter_context(tc.tile_pool(name="epool", bufs=3))
        wbcp = ctx.enter_context(tc.tile_pool(name="wbcp", bufs=2))
        smx = ctx.enter_context(tc.tile_pool(name="smx", bufs=2))
        dramp = ctx.enter_context(tc.tile_pool(name="dramp", bufs=2, space="DRAM"))
        psum_e = ctx.enter_context(tc.tile_pool(name="psum_e", bufs=4, space="PSUM"))
        psum_s = ctx.enter_context(tc.tile_pool(name="psum_s", bufs=2, space="PSUM"))
        psum_q = ctx.enter_context(tc.tile_pool(name="psum_q", bufs=2, space="PSUM"))

        keysT = ins["keysT"].rearrange("b (c p) s -> p b c s", p=P)

        # Prefetch the first stage-1 keys tile ahead of the (larger)
        # constant loads so the PE pipeline fills as early as possible.
        kt_first = ktp.tile([P, HC, ST], F32R)
        nc.sync.dma_start(out=kt_first, in_=keysT[:, 0, :, 0:ST])

        # ---- constants: weights in chunked [p, chunk, ...] layout.
        # Matmul operand DRAM tensors are declared float32r (same bits as
        # the host's fp32) so plain HWDGE DMAs satisfy the BIR verifier's
        # f32r rounding chain without slow casting SWDGE transfers.
        # ua is loaded in 8 column-block DMAs so stage-1 co=0 can start
        # before the whole 4 MiB tensor lands.
        va_sb = consts.tile([P, HC, 1], F32R)
        nc.sync.dma_start(out=va_sb, in_=ins["va"].rearrange("(c p) o -> p c o", p=P))
        qt_sb = consts.tile([P, HC, BL], F32R)
        nc.sync.dma_start(out=qt_sb, in_=ins["qT"].rearrange("(c p) b -> p c b", p=P))
        bc_sb = consts.tile([P, HC], F32)
        nc.sync.dma_start(out=bc_sb, in_=ins["bcomb"].rearrange("(c p) -> p c", p=P))

        # ---- q_proj for all local batches; bias[p, co, b] = qW[co*P+p, b] + bcomb
        wa_ap = ins["wa"].rearrange("(c p) j -> p c j", p=P)
        bias_sb = consts.tile([P, HC, BL], F32)
        for co in range(HC):
            wa_co = wap.tile([P, HC, P], F32R)
            nc.sync.dma_start(out=wa_co, in_=wa_ap[:, :, co * P : (co + 1) * P])
            pq = psum_q.tile([P, BL], F32)
            for ci in range(HC):
                nc.tensor.matmul(
                    pq,
                    lhsT=wa_co[:, ci, :],
                    rhs=qt_sb[:, ci, :],
                    start=(ci == 0),
                    stop=(ci == HC - 1),
                )
            nc.vector.tensor_scalar_add(bias_sb[:, co, :], pq, bc_sb[:, co : co + 1])

        # ua after the q_proj operands, in 8 column-block DMAs so stage-1
        # co=0 can start before the whole 4 MiB tensor lands
        ua_ap = ins["ua"].rearrange("(c p) j -> p c j", p=P)
        ua_sb = consts.tile([P, HC, H], F32R)
        for co in range(HC):
            nc.sync.dma_start(
                out=ua_sb[:, :, co * P : (co + 1) * P],
                in_=ua_ap[:, :, co * P : (co + 1) * P],
            )

        for b in range(BL):
            # ---- stage 1+2: e^T = tanh(Ua^T K^T + bias); scores = Va^T e^T
            sc_sb = smx.tile([1, S], F32)
            mx4 = smx.tile([1, NST], F32)
            for st in range(NST):
                sl = slice(st * ST, (st + 1) * ST)
                if b == 0 and st == 0:
                    kt = kt_first
                else:
                    kt = ktp.tile([P, HC, ST], F32R)
                    nc.sync.dma_start(out=kt, in_=keysT[:, b, :, sl])
                ps = psum_s.tile([1, ST], F32)
                for co in range(HC):
                    pe = psum_e.tile([P, ST], F32)
                    for ci in range(HC):
                        nc.tensor.matmul(
                            pe,
                            lhsT=ua_sb[:, ci, co * P : (co + 1) * P],
                            rhs=kt[:, ci, :],
                            start=(ci == 0),
                            stop=(ci == HC - 1),
                        )
                    e_sb = epool.tile([P, ST], E_DTYPE)
                    nc.scalar.activation(
                        out=e_sb, in_=pe, func=Tanh, bias=bias_sb[:, co, b : b + 1]
                    )
                    nc.tensor.matmul(
                        ps,
                        lhsT=va_sb[:, co, :],
                        rhs=e_sb,
                        start=(co == 0),
                        stop=(co == HC - 1),
                    )
                nc.vector.tensor_copy(out=sc_sb[:, sl], in_=ps)
                # partial max per s-tile, off the softmax critical path
                nc.vector.tensor_reduce(
                    mx4[:, st : st + 1], ps, axis=mybir.AxisListType.X, op=amax
                )

            # ---- softmax on the [1, S] scores row (fp32, in-place)
            mx = smx.tile([1, 1], F32)
            nc.vector.tensor_reduce(mx, mx4, axis=mybir.AxisListType.X, op=amax)
            neg_mx = smx.tile([1, 1], F32)
            nc.scalar.mul(neg_mx, mx, -1.0)
            ssum = smx.tile([1, 1], F32)
            nc.scalar.activation(
                out=sc_sb, in_=sc_sb, func=Exp, bias=neg_mx, accum_out=ssum
            )

            # un-normalized exp row -> DRAM bounce -> broadcast to 128
            # partitions; the 1/sum scale is applied to the context at the
            # end instead, keeping the reciprocal off stage-3's critical path
            wd = dramp.tile([1, S], F32)
            nc.sync.dma_start(out=wd, in_=sc_sb)
            wbc = wbcp.tile([P, S], F32)
            nc.sync.dma_start(out=wbc, in_=wd.broadcast_to([P, S]))

            rsum = smx.tile([1, 1], F32)
            nc.vector.reciprocal(rsum, ssum)
            rd = dramp.tile([1, 1], F32)
            nc.sync.dma_start(out=rd, in_=rsum)
            rs_bc = smx.tile([P, 1], F32)
            nc.sync.dma_start(out=rs_bc, in_=rd.broadcast_to([P, 1]))

            # normalized weights output row
            nc.vector.tensor_scalar_mul(sc_sb, sc_sb, rsum)
            nc.sync.dma_start(out=outs["w_out"][b : b + 1, :], in_=sc_sb)

            # ---- stage 3: context^T[h] = sum_s keysT[h, s] * wexp[s] / sum
            # keysT re-streamed from DRAM per h-chunk in fp32; fused
            # multiply+reduce in one DVE pass via scalar_tensor_tensor
            ctxt_raw = smx.tile([P, HC], F32)
            for c in range(HC):
                kt3 = kt3p.tile([P, S], F32)
                nc.sync.dma_start(out=kt3, in_=keysT[:, b, c, :].bitcast(F32))
                junk = wbcp.tile([P, S], mybir.dt.bfloat16)
                nc.vector.scalar_tensor_tensor(
                    out=junk,
                    in0=kt3,
                    scalar=1.0,
                    in1=wbc,
                    op0=mult,
                    op1=mult,
                    accum_out=ctxt_raw[:, c : c + 1],
                )
            ctxt = smx.tile([P, HC], F32)
            nc.vector.tensor_scalar_mul(ctxt, ctxt_raw, rs_bc)
            nc.sync.dma_start(
                out=outs["ctx_out"][b].rearrange("(c p) -> p c", p=P), in_=ctxt
            )


def build_nc():
    nc = bacc.Bacc("TRN2", target_bir_lowering=False, debug=False)
    ins = {
        "keysT": nc.dram_tensor("keysT", [BL, H, S], F32R, kind="ExternalInput"),
        "ua": nc.dram_tensor("ua", [H, H], F32R, kind="ExternalInput"),
        "wa": nc.dram_tensor("wa", [H, H], F32R, kind="ExternalInput"),
        "va": nc.dram_tensor("va", [H, 1], F32R, kind="ExternalInput"),
        "qT": nc.dram_tensor("qT", [H, BL], F32R, kind="ExternalInput"),
        "bcomb": nc.dram_tensor("bcomb", [H], F32, kind="ExternalInput"),
    }
    outs = {
        "ctx_out": nc.dram_tensor("ctx_out", [BL, H], F32, kind="ExternalOutput"),
        "w_out": nc.dram_tensor("w_out", [BL, S], F32, kind="ExternalOutput"),
    }
    with tile.TileContext(nc) as tc:
        build_core_program(
            tc,
            {k: v.ap() for k, v in outs.items()},
            {k: v.ap() for k, v in ins.items()},
        )
    nc.compile()
    return nc


_NC_CACHE = []


def prepare_in_maps(query, keys, Wa_w, Wa_b, Ua_w, Ua_b, Va_w, Va_b):
    query = np.asarray(query, dtype=np.float32)
    keys = np.asarray(keys, dtype=np.float32)
    ua_h = np.ascontiguousarray(np.asarray(Ua_w, dtype=np.float32))
    wa_h = np.ascontiguousarray(np.asarray(Wa_w, dtype=np.float32))
    va_h = np.ascontiguousarray(np.asarray(Va_w, dtype=np.float32))
    bcomb = (np.asarray(Wa_b, np.float32) + np.asarray(Ua_b, np.float32)).astype(
        np.float32
    )
    in_maps = []
    for c in range(NCORES):
        bs = slice(c * BL, (c + 1) * BL)
        in_maps.append(
            {
                "keysT": np.ascontiguousarray(keys[bs].transpose(0, 2, 1)),
                "ua": ua_h,
                "wa": wa_h,
                "va": va_h,
                "qT": np.ascontiguousarray(query[bs, 0, :].T),
                "bcomb": bcomb,
            }
        )
    return in_maps


def kernel(query, keys, Wa_w, Wa_b, Ua_w, Ua_b, Va_w, Va_b):
    from concourse.bass_utils import run_bass_kernel_spmd

    if not _NC_CACHE:
        _NC_CACHE.append(build_nc())
    nc = _NC_CACHE[0]
    in_maps = prepare_in_maps(query, keys, Wa_w, Wa_b, Ua_w, Ua_b, Va_w, Va_b)
    res = run_bass_kernel_spmd(nc, in_maps, list(range(NCORES)))
    ctx = np.concatenate([np.asarray(r["ctx_out"]) for r in res.results], axis=0)
    wts = np.concatenate([np.asarray(r["w_out"]) for r in res.results], axis=0)
    return (
        ctx.reshape(B, 1, H).astype(np.float32),
        wts.reshape(B, 1, S).astype(np.float32),
    )


# revision 31
# speedup vs baseline: 1.0242x; 1.0162x over previous
"""Bahdanau attention Trainium2 kernel.

Full-input contract: kernel(**inputs) takes the unsharded inputs
(query [32,1,1024], keys [32,2048,1024], Wa_w/Wa_b/Ua_w/Ua_b/Va_w/Va_b)
and returns (context [32,1,1024], weights [32,1,2048]) as float32,
matching reference().

Strategy: data-parallel over batch across 8 NeuronCores (4 batches per
core). Host pre-transposes keys to [b, h, s] (fp32) so the dominant
matmul k_proj^T = Ua^T @ keys^T streams directly from DRAM with no
on-device transposes. All matmuls run in float32r (full fp32 data,
mantissa-rounded by casting gpsimd DMAs / ACT outputs), which the PE
executes at bf16 speed (1 cycle/row at N=512) with ~13x better
accuracy than bf16. Per core:
  stage 1 (PE):  e^T[h,s] = tanh(Ua^T keys^T + (q W_a + Wa_b + Ua_b))
                 f32r matmuls, fp32 PSUM accum, tanh+bias fused on ACT
  stage 2 (PE):  scores[1,s] = Va^T e^T accumulated over h chunks
  softmax (DVE/ACT): on scores row, fp32
  stage 3 (DVE): context^T[h,1] = sum_s keysT[h,s] * wexp[s] / sum via a
                 single fused scalar_tensor_tensor (multiply+reduce) pass
                 per h-chunk against the DMA-broadcast exp'd scores;
                 keysT is re-streamed from DRAM in fp32
Va_b is dropped: softmax is shift-invariant so it cancels exactly.
"""

import os

os.environ.setdefault("JAX_PLATFORMS", "axon")

import numpy as np
from contextlib import ExitStack

import concourse.bass as bass
import concourse.bacc as bacc
import concourse.tile as tile
from concourse import mybir

B, S, H = 32, 2048, 1024
NCORES = 8
BL = B // NCORES  # batches per core
P = 128
HC = H // P  # h chunks
ST = 512  # s tile for stage 1/2
NST = S // ST
F32 = mybir.dt.float32
F32R = mybir.dt.float32r

E_DTYPE = F32R  # tanh output / stage-2 operand dtype


def build_core_program(tc, outs, ins):
    """Emit the per-core Tile program.

    outs: dict with APs ctx_out [BL, H] f32, w_out [BL, S] f32
    ins:  dict with APs keysT [BL, H, S] f32, ua [H, H] f32,
          wa [H, H] f32, va [H, 1] f32, qT [H, BL] f32,
          bcomb [H] f32 (Wa_b + Ua_b)
    """
    nc = tc.nc
    Tanh = mybir.ActivationFunctionType.Tanh
    Exp = mybir.ActivationFunctionType.Exp
    add = mybir.AluOpType.add
    mult = mybir.AluOpType.mult
    amax = mybir.AluOpType.max

    # Sim harness declares plain-f32 DRAM inputs; view them as f32r so
    # HWDGE DMAs into f32r tiles are cast-free in both paths.
    ins = dict(ins)
    for name in ("keysT", "ua", "wa", "va", "qT"):
        if ins[name].dtype != F32R:
            ins[name] = ins[name].bitcast(F32R)

    with ExitStack() as ctx:
        consts = ctx.enter_context(tc.tile_pool(name="consts", bufs=1))
        wap = ctx.enter_context(tc.tile_pool(name="wap", bufs=2))
        ktp = ctx.enter_context(tc.tile_pool(name="ktp", bufs=3))
        kt3p = ctx.enter_context(tc.tile_pool(name="kt3p", bufs=3))
        epool = ctx.enter_context(tc.tile_pool(name="epool", bufs=3))
        wbcp = ctx.enter_context(tc.tile_pool(name="wbcp", bufs=2))
        smx = ctx.enter_context(tc.tile_pool(name="smx", bufs=2))
        dramp = ctx.enter_context(tc.tile_pool(name="dramp", bufs=2, space="DRAM"))
        psum_e = ctx.enter_context(tc.tile_pool(name="psum_e", bufs=4, space="PSUM"))
        psum_s = ctx.enter_context(tc.tile_pool(name="psum_s", bufs=2, space="PSUM"))
        psum_q = ctx.enter_context(tc.tile_pool(name="psum_q", bufs=2, space="PSUM"))

        keysT = ins["keysT"].rearrange("b (c p) s -> p b c s", p=P)

        # Prefetch the first stage-1 keys tile ahead of the (larger)
        # constant loads so the PE pipeline fills as early as possible.
        kt_first = ktp.tile([P, HC, ST], F32R)
        nc.sync.dma_start(out=kt_first, in_=keysT[:, 0, :, 0:ST])

        # ---- constants: weights in chunked [p, chunk, ...] layout.
        # Matmul operand DRAM tensors are declared float32r (same bits as
        # the host's fp32) so plain HWDGE DMAs satisfy the BIR verifier's
        # f32r rounding chain without slow casting SWDGE transfers.
        # ua is loaded in 8 column-block DMAs so stage-1 co=0 can start
        # before the whole 4 MiB tensor lands.
        va_sb = consts.tile([P, HC, 1], F32R)
        nc.sync.dma_start(out=va_sb, in_=ins["va"].rearrange("(c p) o -> p c o", p=P))
        qt_sb = consts.tile([P, HC, BL], F32R)
        nc.sync.dma_start(out=qt_sb, in_=ins["qT"].rearrange("(c p) b -> p c b", p=P))
        bc_sb = consts.tile([P, HC], F32)
        nc.sync.dma_start(out=bc_sb, in_=ins["bcomb"].rearrange("(c p) -> p c", p=P))

        # ---- q_proj for all local batches; bias[p, co, b] = qW[co*P+p, b] + bcomb
        wa_ap = ins["wa"].rearrange("(c p) j -> p c j", p=P)
        bias_sb = consts.tile([P, HC, BL], F32)
        for co in range(HC):
            wa_co = wap.tile([P, HC, P], F32R)
            nc.sync.dma_start(out=wa_co, in_=wa_ap[:, :, co * P : (co + 1) * P])
            pq = psum_q.tile([P, BL], F32)
            for ci in range(HC):
                nc.tensor.matmul(
                    pq,
                    lhsT=wa_co[:, ci, :],
                    rhs=qt_sb[:, ci, :],
                    start=(ci == 0),
                    stop=(ci == HC - 1),
                )
            nc.vector.tensor_scalar_add(bias_sb[:, co, :], pq, bc_sb[:, co : co + 1])

        # ua after the q_proj operands, in 8 column-block DMAs so stage-1
        # co=0 can start before the whole 4 MiB tensor lands
        ua_ap = ins["ua"].rearrange("(c p) j -> p c j", p=P)
        ua_sb = consts.tile([P, HC, H], F32R)
        for co in range(HC):
            nc.sync.dma_start(
                out=ua_sb[:, :, co * P : (co + 1) * P],
                in_=ua_ap[:, :, co * P : (co + 1) * P],
            )

        for b in range(BL):
            # ---- stage 1+2: e^T = tanh(Ua^T K^T + bias); scores = Va^T e^T
            sc_sb = smx.tile([1, S], F32)
            mx4 = smx.tile([1, NST], F32)
            for st in range(NST):
                sl = slice(st * ST, (st + 1) * ST)
                if b == 0 and st == 0:
                    kt = kt_first
                else:
                    kt = ktp.tile([P, HC, ST], F32R)
                    nc.sync.dma_start(out=kt, in_=keysT[:, b, :, sl])
                ps = psum_s.tile([1, ST], F32)
                for co in range(HC):
                    pe = psum_e.tile([P, ST], F32)
                    for ci in range(HC):
                        nc.tensor.matmul(
                            pe,
                            lhsT=ua_sb[:, ci, co * P : (co + 1) * P],
                            rhs=kt[:, ci, :],
                            start=(ci == 0),
                            stop=(ci == HC - 1),
                        )
                    e_sb = epool.tile([P, ST], E_DTYPE)
                    nc.scalar.activation(
                        out=e_sb, in_=pe, func=Tanh, bias=bias_sb[:, co, b : b + 1]
                    )
                    nc.tensor.matmul(
                        ps,
                        lhsT=va_sb[:, co, :],
                        rhs=e_sb,
                        start=(co == 0),
                        stop=(co == HC - 1),
                    )
                nc.vector.tensor_copy(out=sc_sb[:, sl], in_=ps)
                # partial max per s-tile, off the softmax critical path
                nc.vector.tensor_reduce(
                    mx4[:, st : st + 1], ps, axis=mybir.AxisListType.X, op=amax
                )

            # ---- softmax on the [1, S] scores row (fp32, in-place)
            mx = smx.tile([1, 1], F32)
            nc.vector.tensor_reduce(mx, mx4, axis=mybir.AxisListType.X, op=amax)
            neg_mx = smx.tile([1, 1], F32)
            nc.scalar.mul(neg_mx, mx, -1.0)
            ssum = smx.tile([1, 1], F32)
            nc.scalar.activation(
                out=sc_sb, in_=sc_sb, func=Exp, bias=neg_mx, accum_out=ssum
            )

            # un-normalized exp row -> DRAM bounce -> broadcast to 128
            # partitions; the 1/sum scale is applied to the context at the
            # end instead, keeping the reciprocal off stage-3's critical path
            wd = dramp.tile([1, S], F32)
            nc.sync.dma_start(out=wd, in_=sc_sb)
            wbc = wbcp.tile([P, S], F32)
            nc.sync.dma_start(out=wbc, in_=wd.broadcast_to([P, S]))

            rsum = smx.tile([1, 1], F32)
            nc.vector.reciprocal(rsum, ssum)
            rd = dramp.tile([1, 1], F32)
            nc.sync.dma_start(out=rd, in_=rsum)
            rs_bc = smx.tile([P, 1], F32)
            nc.sync.dma_start(out=rs_bc, in_=rd.broadcast_to([P, 1]))

            # normalized weights output row
            nc.vector.tensor_scalar_mul(sc_sb, sc_sb, rsum)
            nc.sync.dma_start(out=outs["w_out"][b : b + 1, :], in_=sc_sb)

            # ---- stage 3: context^T[h] = sum_s keysT[h, s] * wexp[s] / sum
            # keysT re-streamed from DRAM per h-chunk in fp32; fused
            # multiply+reduce in one DVE pass via scalar_tensor_tensor
            ctxt_raw = smx.tile([P, HC], F32)
            for c in range(HC):
                kt3 = kt3p.tile([P, S], F32)
                nc.sync.dma_start(out=kt3, in_=keysT[:, b, c, :].bitcast(F32))
                junk = wbcp.tile([P, S], mybir.dt.bfloat16)
                nc.vector.scalar_tensor_tensor(
                    out=junk,
                    in0=kt3,
                    scalar=1.0,
                    in1=wbc,
                    op0=mult,
                    op1=mult,
                    accum_out=ctxt_raw[:, c : c + 1],
                )
            ctxt = smx.tile([P, HC], F32)
            nc.vector.tensor_scalar_mul(ctxt, ctxt_raw, rs_bc)
            nc.sync.dma_start(
                out=outs["ctx_out"][b].rearrange("(c p) -> p c", p=P), in_=ctxt
            )


def build_nc():
    nc = bacc.Bacc("TRN2", target_bir_lowering=False, debug=False)
    ins = {
        "keysT": nc.dram_tensor("keysT", [BL, H, S], F32R, kind="ExternalInput"),
        "ua": nc.dram_tensor("ua", [H, H], F32R, kind="ExternalInput"),
        "wa": nc.dram_tensor("wa", [H, H], F32R, kind="ExternalInput"),
        "va": nc.dram_tensor("va", [H, 1], F32R, kind="ExternalInput"),
        "qT": nc.dram_tensor("qT", [H, BL], F32R, kind="ExternalInput"),
        "bcomb": nc.dram_tensor("bcomb", [H], F32, kind="ExternalInput"),
    }
    outs = {
        "ctx_out": nc.dram_tensor("ctx_out", [BL, H], F32, kind="ExternalOutput"),
        "w_out": nc.dram_tensor("w_out", [BL, S], F32, kind="ExternalOutput"),
    }
    with tile.TileContext(nc) as tc:
        build_core_program(
            tc,
            {k: v.ap() for k, v in outs.items()},
            {k: v.ap() for k, v in ins.items()},
        )
    nc.compile()
    return nc


_NC_CACHE = []


def prepare_in_maps(query, keys, Wa_w, Wa_b, Ua_w, Ua_b, Va_w, Va_b):
    query = np.asarray(query, dtype=np.float32)
    keys = np.asarray(keys, dtype=np.float32)
    ua_h = np.ascontiguousarray(np.asarray(Ua_w, dtype=np.float32))
    wa_h = np.ascontiguousarray(np.asarray(Wa_w, dtype=np.float32))
    va_h = np.ascontiguousarray(np.asarray(Va_w, dtype=np.float32))
    bcomb = (np.asarray(Wa_b, np.float32) + np.asarray(Ua_b, np.float32)).astype(
        np.float32
    )
    in_maps = []
    for c in range(NCORES):
        bs = slice(c * BL, (c + 1) * BL)
        in_maps.append(
            {
                "keysT": np.ascontiguousarray(keys[bs].transpose(0, 2, 1)),
                "ua": ua_h,
                "wa": wa_h,
                "va": va_h,
                "qT": np.ascontiguousarray(query[bs, 0, :].T),
                "bcomb": bcomb,
            }
        )
    return in_maps


def kernel(query, keys, Wa_w, Wa_b, Ua_w, Ua_b, Va_w, Va_b):
    from concourse.bass_utils import run_bass_kernel_spmd

    if not _NC_CACHE:
        _NC_CACHE.append(build_nc())
    nc = _NC_CACHE[0]
    in_maps = prepare_in_maps(query, keys, Wa_w, Wa_b, Ua_w, Ua_b, Va_w, Va_b)
    res = run_bass_kernel_spmd(nc, in_maps, list(range(NCORES)))
    ctx = np.concatenate([np.asarray(r["ctx_out"]) for r in res.results], axis=0)
    wts = np.concatenate([np.asarray(r["w_out"]) for r in res.results], axis=0)
    return (
        ctx.reshape(B, 1, H).astype(np.float32),
        wts.reshape(B, 1, S).astype(np.float32),
    )
